# revision 1
# baseline (speedup 1.0000x reference)
"""Multi-head self-attention (1x1-conv QKV -> softmax attention -> 1x1-conv)
on Trainium2, 8 NeuronCores, data-parallel over (batch, query-half).

Problem (hardcoded): x[4,256,48,48], Wqkv[768,256], bqkv[768], W0[256,256],
b0[256]; heads=8, dim_head=32, n=2304 pixels.

Sharding: core = b*2 + half. Each core computes K/V for its whole image
(2304 keys) and attention + output projection for its 1152 queries.
No cross-core communication.

Per-core dataflow (all fp32):
  - x_aug [257, 2304]: image (query half permuted first) + ones row.
  - k_all [(m,d)=256, j]  = Wk^T-gathered @ x (+bias via ACT Identity)
  - q_all [(m,d)=256, i]  (Wq, bq pre-scaled by d^-0.5 on host)
  - vT    [j, 8*(32+1)]   = x^T @ Wv_aug: per head 32 v-dims + a ones col;
                            bias + ones via the x ones-row (K=257 matmul).
  - scores^T S_T[j, i] = k_m^T q_m per head: K=32 matmuls, heads processed
    in PAIRS, packed into the PE via row tile_position; each matmul's
    output owns a full PSUM bank (hard constraint on this toolchain:
    one matmul group per bank).
  - P = exp(S_T) on ScalarE (no max subtraction needed: |s| <~ 6).
  - out^T+den = [vT | 1]^T @ P: M=33 matmuls col-positioned (0,0)/(0,64)
    into two separate PSUM banks; accumulation over the 18 key tiles.
    Row 32 (head0) / 96 (head1) = softmax denominators.
  - normalize: den rows copied to partition 0 (mixed-base tensor_copy),
    reciprocal there (custom DVE ops read the input tensor's partition 0),
    stream_shuffle broadcast across the 32-block, DVE multiply into outc.
  - y = W0 @ outc + b0 with W0 host-arranged to outc's partition layout.
"""

import os as _os

import numpy as np

import concourse.bass as bass
import concourse.mybir as mybir
import concourse.tile as tile
from concourse import bacc
from concourse import bass_utils

F32 = mybir.dt.float32
BF16 = mybir.dt.bfloat16
AF = mybir.ActivationFunctionType

B, C, HH, WW = 4, 256, 48, 48
HEADS, D = 8, 32
N = HH * WW            # 2304 keys per image
NCORES = 8
NQ = N // 2            # 1152 queries per core
JT = N // 128          # 18 key tiles
ICW = 384              # query chunk width (3 chunks per core)
NV = HEADS * (D + 1)   # 264: vT columns (32 v dims + ones col per head)

DEBUG_STAGE = int(_os.environ.get("KSTAGE", "4"))
RECIP_MODE = _os.environ.get("KREC", "fast")
QK_DT = BF16 if _os.environ.get("KQK", "f32") == "bf16" else F32
PV_DT = BF16 if _os.environ.get("KPV", "f32") == "bf16" else F32
MM_CAST = _os.environ.get("KMM", "f32")   # "f32r": bitcast attention matmul
                                          # operands to float32r (single-pass PE)


def _mm(ap):
    if MM_CAST == "f32r" and ap.dtype == F32:
        return ap.bitcast(mybir.dt.float32r)
    return ap


def _chunks(total, step):
    out = []
    o = 0
    while o < total:
        w = min(step, total - o)
        out.append((o, w))
        o += w
    return out


def _body(tc, x_d, wq_d, bq_d, wk_d, bk_d, wv_d, w0_d, w0b_d, y_d):
    from contextlib import ExitStack

    nc = tc.nc
    with ExitStack() as ctx:
        const = ctx.enter_context(tc.tile_pool(name="const", bufs=1))
        data = ctx.enter_context(tc.tile_pool(name="data", bufs=1))

        # ---------------- load inputs ----------------
        x_sb = [const.tile([128, N], F32, name=f"xa{t}", tag=f"xa{t}") for t in range(2)]
        x1_sb = const.tile([1, N], F32, name="xones", tag="xones")
        nc.sync.dma_start(x_sb[0][:], x_d[0:128, :])
        nc.sync.dma_start(x_sb[1][:], x_d[128:256, :])
        nc.gpsimd.dma_start(x1_sb[:], x_d[256:257, :])

        def load2(name, dram, cols):
            ts_ = [const.tile([128, cols], F32, name=f"{name}{t}", tag=f"{name}{t}") for t in range(2)]
            nc.sync.dma_start(ts_[0][:], dram[0:128, :])
            nc.sync.dma_start(ts_[1][:], dram[128:256, :])
            return ts_

        wq_sb = load2("wq", wq_d, C)
        wk_sb = load2("wk", wk_d, C)
        wv_sb = load2("wv", wv_d, NV)
        wv1_sb = const.tile([1, NV], F32, name="wvbias", tag="wvbias")
        nc.gpsimd.dma_start(wv1_sb[:], wv_d[256:257, :])
        w0_sb = load2("w0", w0_d, C)
        w01_sb = const.tile([1, C], F32, name="w0bias", tag="w0bias")
        nc.gpsimd.dma_start(w01_sb[:], w0b_d[0:1, :])
        bq_sb = load2("bq", bq_d, 1)
        bk_sb = load2("bk", bk_d, 1)

        ones_row = const.tile([1, NQ], F32, name="ones_row", tag="ones_row")
        nc.vector.memset(ones_row[:], 1.0)

        # persistent activations
        k_sb = [data.tile([128, N], QK_DT, name=f"k{g}", tag=f"k{g}") for g in range(2)]
        q_sb = [data.tile([128, NQ], QK_DT, name=f"q{g}", tag=f"q{g}") for g in range(2)]
        vt_sb = [data.tile([128, NV], PV_DT, name=f"vt{j}", tag=f"vt{j}") for j in range(JT)]
        # output tiles in pv layout: tile t = hg*2 + pr holds head 4*hg+2*pr
        # at partitions 0-31 and head 4*hg+2*pr+1 at partitions 64-95
        outc_sb = [data.tile([128, NQ], F32, name=f"oc{t}", tag=f"oc{t}") for t in range(4)]
        y_sb = [data.tile([128, NQ], F32, name=f"y{g}", tag=f"y{g}") for g in range(2)]

        # ---------------- projections ----------------
        with tc.tile_pool(name="prj", bufs=2, space="PSUM") as prj:
            for hg in range(2):
                hsl = slice(hg * 128, (hg + 1) * 128)
                for (o, w) in _chunks(N, 512):
                    kps = prj.tile([128, 512], F32, name="kps", tag="kps")
                    nc.tensor.matmul(kps[:, :w], wk_sb[0][:, hsl], x_sb[0][:, o:o + w], start=True, stop=False)
                    nc.tensor.matmul(kps[:, :w], wk_sb[1][:, hsl], x_sb[1][:, o:o + w], start=False, stop=True)
                    nc.scalar.activation(k_sb[hg][:, o:o + w], kps[:, :w], AF.Identity, bias=bk_sb[hg][:, 0:1])
                for (o, w) in _chunks(NQ, 512):
                    qps = prj.tile([128, 512], F32, name="qps", tag="qps")
                    nc.tensor.matmul(qps[:, :w], wq_sb[0][:, hsl], x_sb[0][:, o:o + w], start=True, stop=False)
                    nc.tensor.matmul(qps[:, :w], wq_sb[1][:, hsl], x_sb[1][:, o:o + w], start=False, stop=True)
                    nc.scalar.activation(q_sb[hg][:, o:o + w], qps[:, :w], AF.Identity, bias=bq_sb[hg][:, 0:1])
            for j in range(JT):
                jsl = slice(j * 128, (j + 1) * 128)
                vps = prj.tile([128, NV], F32, name="vps", tag="vps")
                nc.tensor.matmul(vps[:], x_sb[0][:, jsl], wv_sb[0][:], start=True, stop=False)
                nc.tensor.matmul(vps[:], x_sb[1][:, jsl], wv_sb[1][:], start=False, stop=False)
                nc.tensor.matmul(vps[:], x1_sb[:, jsl], wv1_sb[:], start=False, stop=True)
                nc.vector.tensor_copy(vt_sb[j][:], vps[:])

        if DEBUG_STAGE < 2:
            for g in range(2):
                nc.vector.tensor_copy(y_sb[g][:], q_sb[g][:])
                nc.sync.dma_start(y_d[g * 128:(g + 1) * 128, :], y_sb[g][:])
            return

        # ---------------- attention main loop ----------------
        # PSUM budget: st 2 bufs x 2 banks + pv0/pv1 2 bufs x 1 bank = 8.
        with tc.tile_pool(name="stp", bufs=2, space="PSUM") as stp, \
             tc.tile_pool(name="pv0p", bufs=2, space="PSUM") as pv0p, \
             tc.tile_pool(name="pv1p", bufs=2, space="PSUM") as pv1p, \
             tc.tile_pool(name="ptp", bufs=3) as ptp, \
             tc.tile_pool(name="epi", bufs=2) as epi:
            for hg in range(2):
                for pr in range(2):
                    rb = pr * 64       # partition base of this head pair
                    t_idx = hg * 2 + pr
                    for (ic0, w) in _chunks(NQ, ICW):
                        pv0 = pv0p.tile([128, ICW], F32, name="pv0", tag="pv0")
                        pv1 = pv1p.tile([128, ICW], F32, name="pv1", tag="pv1")
                        pts = {}

                        def emit_pv(j, w=w, pv0=pv0, pv1=pv1, pts=pts, hg=hg, pr=pr):
                            pt = pts.pop(j)
                            for hl, (pv, base) in enumerate(((pv0, 0), (pv1, 64))):
                                gh = hg * 4 + 2 * pr + hl
                                nc.tensor.matmul(
                                    pv[base:base + 33, 0:w],
                                    _mm(vt_sb[j][:, gh * 33:gh * 33 + 33]),
                                    _mm(pt[:, hl * ICW:hl * ICW + w]),
                                    start=(j == 0), stop=(j == JT - 1),
                                    tile_position=(0, base),
                                )

                        for j in range(JT):
                            st = stp.tile([128, 1024], F32, name="st", tag="st")
                            for hl in range(2):
                                nc.tensor.matmul(
                                    st[:, hl * 512:hl * 512 + w],
                                    _mm(k_sb[hg][rb + hl * 32:rb + (hl + 1) * 32, j * 128:(j + 1) * 128]),
                                    _mm(q_sb[hg][rb + hl * 32:rb + (hl + 1) * 32, ic0:ic0 + w]),
                                    start=True, stop=True,
                                    tile_position=(rb + hl * 32, 0),
                                )
                            pt = ptp.tile([128, 2 * ICW], PV_DT, name="pt", tag="pt")
                            nc.scalar.activation(
                                pt[:].rearrange("p (s q) -> p s q", s=2),
                                st[:].rearrange("p (s q) -> p s q", s=2)[:, :, 0:w],
                                AF.Exp,
                            )
                            pts[j] = pt
                            if j >= 1:
                                emit_pv(j - 1)
                        emit_pv(JT - 1)

                        # epilogue: denominators live at psum partition 32
                        # (head 0, pv0) / 96 (head 1, pv1).  reciprocal in
                        # place, round-trip through DRAM, partition-broadcast,
                        # multiply into outc (all ops base-aligned).
                        oc = outc_sb[t_idx]
                        if DEBUG_STAGE < 3:
                            nc.vector.tensor_copy(oc[0:33, ic0:ic0 + w], pv0[0:33, 0:w])
                            nc.vector.tensor_copy(oc[64:97, ic0:ic0 + w], pv1[64:97, 0:w])
                            continue

                        # HW-verified chain: copy each den row to partition 0 of
                        # its own tile (mixed-base tensor_copy works; custom DVE
                        # ops read the input tensor's partition 0 regardless of
                        # the AP base), reciprocal there, broadcast across the
                        # 32-block with stream_shuffle (mask of zeros), shift
                        # head1's block to base 64 with another copy, multiply.
                        dt0 = epi.tile([1, ICW], F32, name="dt0", tag="dt0")
                        dt1 = epi.tile([1, ICW], F32, name="dt1", tag="dt1")
                        nc.vector.tensor_copy(dt0[0:1, 0:w], pv0[32:33, 0:w])
                        nc.vector.tensor_copy(dt1[0:1, 0:w], pv1[96:97, 0:w])
                        rt0 = epi.tile([32, ICW], F32, name="rt0", tag="rt0")
                        rt1 = epi.tile([32, ICW], F32, name="rt1", tag="rt1")
                        nc.vector.memset(rt0[:, 0:w], 1.0)
                        nc.vector.memset(rt1[:, 0:w], 1.0)
                        if RECIP_MODE == "fast":
                            nc.vector.reciprocal_approx_fast(rt0[0:1, 0:w], dt0[0:1, 0:w])
                            nc.vector.reciprocal_approx_fast(rt1[0:1, 0:w], dt1[0:1, 0:w])
                        else:
                            nc.vector.reciprocal(rt0[0:1, 0:w], dt0[0:1, 0:w])
                            nc.vector.reciprocal(rt1[0:1, 0:w], dt1[0:1, 0:w])
                        rr = epi.tile([128, ICW], F32, name="rr", tag="rr")
                        rrb = epi.tile([32, ICW], F32, name="rrb", tag="rrb")
                        nc.vector.stream_shuffle(rr[0:32, 0:w], rt0[0:32, 0:w], [0] * 32)
                        nc.vector.stream_shuffle(rrb[0:32, 0:w], rt1[0:32, 0:w], [0] * 32)
                        nc.vector.tensor_copy(rr[64:96, 0:w], rrb[0:32, 0:w])
                        nc.vector.tensor_mul(oc[0:32, ic0:ic0 + w], pv0[0:32, 0:w], rr[0:32, 0:w])
                        nc.vector.tensor_mul(oc[64:96, ic0:ic0 + w], pv1[64:96, 0:w], rr[64:96, 0:w])

        if DEBUG_STAGE < 4:
            for g in range(2):
                nc.sync.dma_start(y_d[g * 128:(g + 1) * 128, :], outc_sb[g][:])
            return

        # ---------------- output projection ----------------
        # compact pv-layout outc tiles into dense head-major [c, i] tiles via
        # SBUF->SBUF DMA partition remap, then plain K=128 matmuls.
        od_sb = [data.tile([128, NQ], F32, name=f"od{g}", tag=f"od{g}") for g in range(2)]
        for g in range(2):
            for pr in range(2):
                src = outc_sb[g * 2 + pr]
                nc.sync.dma_start(od_sb[g][pr * 64:pr * 64 + 32, :], src[0:32, :])
                nc.sync.dma_start(od_sb[g][pr * 64 + 32:pr * 64 + 64, :], src[64:96, :])
        with tc.tile_pool(name="fin", bufs=2, space="PSUM") as fin:
            for mt in range(2):
                msl = slice(mt * 128, (mt + 1) * 128)
                for (o, w) in _chunks(NQ, 512):
                    fps = fin.tile([128, 512], F32, name="fps", tag="fps")
                    nc.tensor.matmul(fps[:, :w], w0_sb[0][:, msl], od_sb[0][:, o:o + w], start=True, stop=False)
                    nc.tensor.matmul(fps[:, :w], w0_sb[1][:, msl], od_sb[1][:, o:o + w], start=False, stop=False)
                    nc.tensor.matmul(fps[:, :w], w01_sb[:, msl], ones_row[:, o:o + w], start=False, stop=True)
                    nc.scalar.copy(y_sb[mt][:, o:o + w], fps[:, :w])
                nc.sync.dma_start(y_d[msl, :], y_sb[mt][:])


def build_program():
    nc = bacc.Bacc(
        "TRN2",
        target_bir_lowering=False,
        debug=False,
        enable_asserts=False,
        num_devices=NCORES,
    )
    x_d = nc.dram_tensor("x", [C + 1, N], F32, kind="ExternalInput").ap()
    wq_d = nc.dram_tensor("wq", [C, C], F32, kind="ExternalInput").ap()
    bq_d = nc.dram_tensor("bq", [C, 1], F32, kind="ExternalInput").ap()
    wk_d = nc.dram_tensor("wk", [C, C], F32, kind="ExternalInput").ap()
    bk_d = nc.dram_tensor("bk", [C, 1], F32, kind="ExternalInput").ap()
    wv_d = nc.dram_tensor("wv", [C + 1, NV], F32, kind="ExternalInput").ap()
    w0_d = nc.dram_tensor("w0", [C, C], F32, kind="ExternalInput").ap()
    w0b_d = nc.dram_tensor("w0b", [1, C], F32, kind="ExternalInput").ap()
    y_d = nc.dram_tensor("y", [C, NQ], F32, kind="ExternalOutput").ap()

    with tile.TileContext(nc) as tc:
        _body(tc, x_d, wq_d, bq_d, wk_d, bk_d, wv_d, w0_d, w0b_d, y_d)
    nc.compile()
    return nc


_CACHE = {}


def _get_program():
    if "nc" not in _CACHE:
        _CACHE["nc"] = build_program()
    return _CACHE["nc"]


def make_in_maps(x, Wqkv, bqkv, W0, b0):
    f = np.float32
    x = np.asarray(x, f)
    Wqkv = np.asarray(Wqkv, f)
    bqkv = np.asarray(bqkv, f)
    W0 = np.asarray(W0, f)
    b0 = np.asarray(b0, f)

    scale = f(D) ** f(-0.5)
    # channel o = d*24 + k*8 + m ; column layout is head-major (m, d) -> m*32+d
    md = (np.arange(HEADS)[:, None] + 24 * np.arange(D)[None, :]).reshape(-1)
    q_rows, k_rows, v_rows = md + 0, md + 8, md + 16

    wq = np.ascontiguousarray((Wqkv[q_rows, :] * scale).T, dtype=f)
    bq = np.ascontiguousarray((bqkv[q_rows] * scale).reshape(-1, 1), dtype=f)
    wk = np.ascontiguousarray(Wqkv[k_rows, :].T, dtype=f)
    bk = np.ascontiguousarray(bqkv[k_rows].reshape(-1, 1), dtype=f)

    wv = np.zeros((C + 1, NV), f)
    for m in range(HEADS):
        vr = v_rows[m * D:(m + 1) * D]
        wv[0:C, m * 33:m * 33 + 32] = Wqkv[vr, :].T
        wv[C, m * 33:m * 33 + 32] = bqkv[vr]
        wv[C, m * 33 + 32] = 1.0

    w0 = np.ascontiguousarray(W0.T, dtype=f)  # [c, o], c rows head-major
    w0b = np.ascontiguousarray(b0[None, :], dtype=f)

    shared = {"wq": wq, "bq": bq, "wk": wk, "bk": bk, "wv": wv, "w0": w0, "w0b": w0b}
    maps = []
    for b in range(B):
        xb = x[b].reshape(C, N)
        for half in range(2):
            if half == 0:
                xp = xb
            else:
                xp = np.concatenate([xb[:, NQ:], xb[:, :NQ]], axis=1)
            x_aug = np.concatenate([xp, np.ones((1, N), f)], axis=0)
            maps.append({"x": np.ascontiguousarray(x_aug), **shared})
    return maps


def assemble_output(ys):
    out = np.empty((B, C, N), np.float32)
    for b in range(B):
        out[b][:, 0:NQ] = ys[2 * b]
        out[b][:, NQ:] = ys[2 * b + 1]
    return out.reshape(B, C, HH, WW)


def run(inputs, trace=False):
    nc = _get_program()
    maps = make_in_maps(**inputs)
    res = bass_utils.run_bass_kernel_spmd(
        nc, maps, core_ids=list(range(NCORES)), trace=trace
    )
    ys = [res.results[c]["y"] for c in range(NCORES)]
    return assemble_output(ys), res.exec_time_ns


def kernel(**inputs):
    out, _ = run(inputs, trace=False)
    return out



# revision 6
# speedup vs baseline: 1.5730x; 1.5730x over previous
"""Multi-head self-attention (1x1-conv QKV -> softmax attention -> 1x1-conv)
on Trainium2, 8 NeuronCores, data-parallel over (batch, query-half).

Problem (hardcoded): x[4,256,48,48], Wqkv[768,256], bqkv[768], W0[256,256],
b0[256]; heads=8, dim_head=32, n=2304 pixels.

Sharding: core = b*2 + half. Each core computes K/V for its whole image
(2304 keys) and attention + output projection for its 1152 queries.
No cross-core communication.

Per-core dataflow — all large matmuls in BF16 (1 cyc/row on the PE vs 4
for fp32; tolerance is 2e-2 so bf16 inputs are plenty):
  - x_aug [257, 2304] bf16: image (query half permuted first) + ones row,
    DMA'd in 512-col chunks so projections start early.
  - k_all [(m,d)=256, j], q_all [(m,d)=256, i] bf16 (Wq, bq pre-scaled by
    d^-0.5 on host), vT [j, 8*(32+1)] bf16: per head 32 v-dims + ones col
    (bias + softmax denominator via the x ones-row / vt ones-col tricks).
  - scores^T S_T[j, i] per head pair: K=32 bf16 matmuls row-packed via
    tile_position; each matmul output owns a full PSUM bank.
  - P = exp(S_T), split across TWO engines per key-tile j:
      * ACT: table exp (exact), bf16 out
      * DVE: Schraudolph bit-trick exp targeting bf16 bits: one
        scalar_tensor_tensor (st*A16 + B16) -> int16 tile, bitcast bf16
        (max elementwise err ~3.4%; mostly cancels post-softmax).
  - out^T+den: the head PAIR's PV matmuls share ONE PSUM bank: a K=1
    "opener" matmul (start=True) writes a pattern row — 0.0 on the output
    rows 0:33/64:97, 1.0 on the junk rows — so both heads can accumulate
    with start=False (pending-zero bytes make their first write an
    overwrite) and the junk rows stay reciprocal-safe. All MMs of a group
    are chained with chain_iter_dep so the scheduler cannot move an
    accumulate past the closing stop.
  - normalize: dens sit at psum partitions 32/96. Two 32-row block copies
    move [den, 1s] blocks to a base-0 tile, one reciprocal_approx_fast
    over 64 partitions, then a K=64 fp32 mask matmul broadcasts recipA to
    partitions 0:32 and recipB to 64:96 of an rr PSUM bank (mask rows
    are zero elsewhere, so the 1.0-recips don't contribute); one full
    [128,w] DVE mul writes normalized bf16 outc.
  - y = W0 @ outc + b0 (bf16), fp32 out, DMA per 512-col chunk.
"""

import os as _os

import numpy as np
import ml_dtypes

import concourse.bass as bass
import concourse.mybir as mybir
import concourse.tile as tile
from concourse import bacc
from concourse import bass_utils

F32 = mybir.dt.float32
BF = mybir.dt.bfloat16
I16 = mybir.dt.int16
AF = mybir.ActivationFunctionType
ALU = mybir.AluOpType
NPBF = ml_dtypes.bfloat16

B, C, HH, WW = 4, 256, 48, 48
HEADS, D = 8, 32
N = HH * WW            # 2304 keys per image
NCORES = 8
NQ = N // 2            # 1152 queries per core
JT = N // 128          # 18 key tiles
ICW = 384              # query chunk width (3 chunks per core)
NV = HEADS * (D + 1)   # 264: vT columns (32 v dims + ones col per head)

# Schraudolph exp in bf16 bit space: exp(s) ~ bitcast_bf16(int16(A16*s + B16))
A16_SCHR = float((1 << 7) / np.log(2.0))         # 184.665
B16_SCHR = float((127 << 7) - 5.375)             # C16=5.375: ~3.4% max elem err

N_DVE_J = int(_os.environ.get("KDVE", "8"))      # of JT=18 key tiles on DVE
DVE_JS = frozenset(((2 * i + 1) * JT) // (2 * N_DVE_J) for i in range(N_DVE_J))


def _chunks(total, step):
    out = []
    o = 0
    while o < total:
        w = min(step, total - o)
        out.append((o, w))
        o += w
    return out


def _body(tc, x_d, wq_d, bq_d, wk_d, bk_d, wv_d, w0_d, w0b_d, msk_d, y_d):
    from contextlib import ExitStack

    nc = tc.nc
    with ExitStack() as ctx:
        const = ctx.enter_context(tc.tile_pool(name="const", bufs=1))
        data = ctx.enter_context(tc.tile_pool(name="data", bufs=1))

        # ---------------- load inputs ----------------
        x_sb = [const.tile([128, N], BF, name=f"xa{t}", tag=f"xa{t}") for t in range(2)]
        x1_sb = const.tile([1, N], BF, name="xones", tag="xones")
        for (o, w) in _chunks(N, 512):
            nc.sync.dma_start(x_sb[0][:, o:o + w], x_d[0:128, o:o + w])
            nc.sync.dma_start(x_sb[1][:, o:o + w], x_d[128:256, o:o + w])
        nc.gpsimd.dma_start(x1_sb[:], x_d[256:257, :])

        def load2(name, dram, cols, dt=BF):
            ts_ = [const.tile([128, cols], dt, name=f"{name}{t}", tag=f"{name}{t}") for t in range(2)]
            nc.sync.dma_start(ts_[0][:], dram[0:128, :])
            nc.sync.dma_start(ts_[1][:], dram[128:256, :])
            return ts_

        wq_sb = load2("wq", wq_d, C)
        wk_sb = load2("wk", wk_d, C)
        wv_sb = load2("wv", wv_d, NV)
        wv1_sb = const.tile([1, NV], BF, name="wvbias", tag="wvbias")
        nc.gpsimd.dma_start(wv1_sb[:], wv_d[256:257, :])
        w0_sb = load2("w0", w0_d, C)
        w01_sb = const.tile([1, C], BF, name="w0bias", tag="w0bias")
        nc.gpsimd.dma_start(w01_sb[:], w0b_d[0:1, :])
        bq_sb = load2("bq", bq_d, 1, dt=F32)
        bk_sb = load2("bk", bk_d, 1, dt=F32)
        mask64 = const.tile([64, 128], F32, name="mask64", tag="mask64")
        nc.gpsimd.dma_start(mask64[:], msk_d[:, :])

        ones_row = const.tile([1, NQ], BF, name="ones_row", tag="ones_row")
        nc.vector.memset(ones_row[:], 1.0)
        # Schraudolph additive constant, matching the exp input AP shape
        bexp = const.tile([128, 2 * ICW], F32, name="bexp", tag="bexp")
        nc.vector.memset(bexp[:], B16_SCHR)
        # bank-opener row: 0 over the PV output rows (0:33, 64:97), 1.0 over
        # the junk rows so the den blocks stay reciprocal-safe
        patt = const.tile([1, 128], BF, name="patt", tag="patt")
        nc.vector.memset(patt[:], 1.0)
        nc.vector.memset(patt[0:1, 0:33], 0.0)
        nc.vector.memset(patt[0:1, 64:97], 0.0)

        # persistent activations
        k_sb = [data.tile([128, N], BF, name=f"k{g}", tag=f"k{g}") for g in range(2)]
        q_sb = [data.tile([128, NQ], BF, name=f"q{g}", tag=f"q{g}") for g in range(2)]
        vt_sb = [data.tile([128, NV], BF, name=f"vt{j}", tag=f"vt{j}") for j in range(JT)]
        # output tiles in pv layout: tile t = hg*2 + pr holds head 4*hg+2*pr
        # at partitions 0-31 and head 4*hg+2*pr+1 at partitions 64-95
        outc_sb = [data.tile([128, NQ], BF, name=f"oc{t}", tag=f"oc{t}") for t in range(4)]
        y_sb = [data.tile([128, NQ], F32, name=f"y{g}", tag=f"y{g}") for g in range(2)]

        # ---------------- projections (bf16) ----------------
        with tc.tile_pool(name="prj", bufs=2, space="PSUM") as prj:
            for hg in range(2):
                hsl = slice(hg * 128, (hg + 1) * 128)
                for (o, w) in _chunks(N, 512):
                    kps = prj.tile([128, 512], F32, name="kps", tag="kps")
                    nc.tensor.matmul(kps[:, :w], wk_sb[0][:, hsl], x_sb[0][:, o:o + w], start=True, stop=False)
                    nc.tensor.matmul(kps[:, :w], wk_sb[1][:, hsl], x_sb[1][:, o:o + w], start=False, stop=True)
                    nc.scalar.activation(k_sb[hg][:, o:o + w], kps[:, :w], AF.Identity, bias=bk_sb[hg][:, 0:1])
                for (o, w) in _chunks(NQ, 512):
                    qps = prj.tile([128, 512], F32, name="qps", tag="qps")
                    nc.tensor.matmul(qps[:, :w], wq_sb[0][:, hsl], x_sb[0][:, o:o + w], start=True, stop=False)
                    nc.tensor.matmul(qps[:, :w], wq_sb[1][:, hsl], x_sb[1][:, o:o + w], start=False, stop=True)
                    nc.scalar.activation(q_sb[hg][:, o:o + w], qps[:, :w], AF.Identity, bias=bq_sb[hg][:, 0:1])
            for j in range(JT):
                jsl = slice(j * 128, (j + 1) * 128)
                vps = prj.tile([128, NV], F32, name="vps", tag="vps")
                nc.tensor.matmul(vps[:], x_sb[0][:, jsl], wv_sb[0][:], start=True, stop=False)
                nc.tensor.matmul(vps[:], x_sb[1][:, jsl], wv_sb[1][:], start=False, stop=False)
                nc.tensor.matmul(vps[:], x1_sb[:, jsl], wv1_sb[:], start=False, stop=True)
                nc.scalar.copy(vt_sb[j][:], vps[:])

        # ---------------- attention main loop ----------------
        # PSUM budget: stp 2x2 banks + pvp 2x1 (bank-shared head pair)
        # + rrp 2x1 = 8.
        with tc.tile_pool(name="stp", bufs=2, space="PSUM") as stp, \
             tc.tile_pool(name="pvp", bufs=2, space="PSUM") as pvp, \
             tc.tile_pool(name="rrp", bufs=2, space="PSUM") as rrp, \
             tc.tile_pool(name="ptp", bufs=4) as ptp, \
             tc.tile_pool(name="epi", bufs=2) as epi:
            for hg in range(2):
                for pr in range(2):
                    rb = pr * 64       # partition base of this head pair
                    t_idx = hg * 2 + pr
                    for (ic0, w) in _chunks(NQ, ICW):
                        ckey = f"pvc{t_idx}_{ic0}"
                        pv = pvp.tile([128, ICW], F32, name="pv", tag="pv")
                        pts = {}

                        # open the shared bank: K=1 matmul writes the pattern
                        # row to all 128 partitions with start=True
                        mi = nc.tensor.matmul(
                            pv[:, 0:w], patt[0:1, 0:128], ones_row[0:1, 0:w],
                            start=True, stop=False, tile_position=(0, 0),
                        )
                        tc.chain_iter_dep(ckey, mi.ins)

                        def emit_pv(j, w=w, pv=pv, pts=pts, hg=hg, pr=pr, ckey=ckey):
                            pt = pts.pop(j)
                            for hl, base in ((0, 0), (1, 64)):
                                gh = hg * 4 + 2 * pr + hl
                                mi = nc.tensor.matmul(
                                    pv[base:base + 33, 0:w],
                                    vt_sb[j][:, gh * 33:gh * 33 + 33],
                                    pt[:].bitcast(BF)[:, hl * ICW:hl * ICW + w],
                                    start=False,
                                    stop=(j == JT - 1 and hl == 1),
                                    tile_position=(0, base),
                                )
                                tc.chain_iter_dep(ckey, mi.ins)

                        for j in range(JT):
                            st = stp.tile([128, 1024], F32, name="st", tag="st")
                            for hl in range(2):
                                nc.tensor.matmul(
                                    st[:, hl * 512:hl * 512 + w],
                                    k_sb[hg][rb + hl * 32:rb + (hl + 1) * 32, j * 128:(j + 1) * 128],
                                    q_sb[hg][rb + hl * 32:rb + (hl + 1) * 32, ic0:ic0 + w],
                                    start=True, stop=True,
                                    tile_position=(rb + hl * 32, 0),
                                )
                            st_v = st[:].rearrange("p (s q) -> p s q", s=2)[:, :, 0:w]
                            if j in DVE_JS:
                                pt = ptp.tile([128, 2 * ICW], I16, name="pt", tag="pt")
                                nc.vector.scalar_tensor_tensor(
                                    pt[:].rearrange("p (s q) -> p s q", s=2)[:, :, 0:w],
                                    st_v, A16_SCHR,
                                    bexp[:].rearrange("p (s q) -> p s q", s=2)[:, :, 0:w],
                                    ALU.mult, ALU.add,
                                )
                            else:
                                pt = ptp.tile([128, 2 * ICW], BF, name="pt", tag="pt")
                                nc.scalar.activation(
                                    pt[:].rearrange("p (s q) -> p s q", s=2)[:, :, 0:w],
                                    st_v, AF.Exp,
                                )
                            pts[j] = pt
                            if j >= 1:
                                emit_pv(j - 1)
                        emit_pv(JT - 1)

                        # epilogue: dens at psum partitions 32 (head 0) and
                        # 96 (head 1), junk rows hold 1.0 from the opener.
                        # Two block copies to a base-0 tile (custom DVE ops
                        # need base 0), one reciprocal over 64 partitions,
                        # K=64 fp32 mask matmul broadcasts the recip rows,
                        # one full-width mul normalizes into bf16 outc.
                        oc = outc_sb[t_idx]
                        dd = epi.tile([64, ICW], F32, name="dd", tag="dd")
                        rc = epi.tile([64, ICW], F32, name="rc", tag="rc")
                        nc.vector.tensor_copy(dd[0:32, 0:w], pv[32:64, 0:w])
                        nc.vector.tensor_copy(dd[32:64, 0:w], pv[96:128, 0:w])
                        nc.vector.reciprocal_approx_fast(rc[0:64, 0:w], dd[0:64, 0:w])
                        rr = rrp.tile([128, ICW], F32, name="rr", tag="rr")
                        nc.tensor.matmul(
                            rr[:, 0:w], mask64[:, :], rc[0:64, 0:w],
                            start=True, stop=True, tile_position=(0, 0),
                        )
                        # DVE reads at most one PSUM operand; stage rr in
                        # SBUF via ACT
                        rs = epi.tile([128, ICW], F32, name="rs", tag="rs")
                        nc.scalar.copy(rs[:, 0:w], rr[:, 0:w])
                        nc.vector.tensor_mul(oc[:, ic0:ic0 + w], pv[:, 0:w], rs[:, 0:w])

        # ---------------- output projection ----------------
        # compact pv-layout outc tiles into dense head-major [c, i] tiles via
        # SBUF->SBUF DMA partition remap, then plain K=128 matmuls.
        od_sb = [data.tile([128, NQ], BF, name=f"od{g}", tag=f"od{g}") for g in range(2)]
        for g in range(2):
            for pr in range(2):
                src = outc_sb[g * 2 + pr]
                nc.sync.dma_start(od_sb[g][pr * 64:pr * 64 + 32, :], src[0:32, :])
                nc.sync.dma_start(od_sb[g][pr * 64 + 32:pr * 64 + 64, :], src[64:96, :])
        with tc.tile_pool(name="fin", bufs=2, space="PSUM") as fin:
            for mt in range(2):
                msl = slice(mt * 128, (mt + 1) * 128)
                for (o, w) in _chunks(NQ, 512):
                    fps = fin.tile([128, 512], F32, name="fps", tag="fps")
                    nc.tensor.matmul(fps[:, :w], w0_sb[0][:, msl], od_sb[0][:, o:o + w], start=True, stop=False)
                    nc.tensor.matmul(fps[:, :w], w0_sb[1][:, msl], od_sb[1][:, o:o + w], start=False, stop=False)
                    nc.tensor.matmul(fps[:, :w], w01_sb[:, msl], ones_row[:, o:o + w], start=False, stop=True)
                    nc.scalar.copy(y_sb[mt][:, o:o + w], fps[:, :w])
                    nc.sync.dma_start(y_d[msl, o:o + w], y_sb[mt][:, o:o + w])


def build_program():
    nc = bacc.Bacc(
        "TRN2",
        target_bir_lowering=False,
        debug=False,
        enable_asserts=False,
        num_devices=NCORES,
    )
    x_d = nc.dram_tensor("x", [C + 1, N], BF, kind="ExternalInput").ap()
    wq_d = nc.dram_tensor("wq", [C, C], BF, kind="ExternalInput").ap()
    bq_d = nc.dram_tensor("bq", [C, 1], F32, kind="ExternalInput").ap()
    wk_d = nc.dram_tensor("wk", [C, C], BF, kind="ExternalInput").ap()
    bk_d = nc.dram_tensor("bk", [C, 1], F32, kind="ExternalInput").ap()
    wv_d = nc.dram_tensor("wv", [C + 1, NV], BF, kind="ExternalInput").ap()
    w0_d = nc.dram_tensor("w0", [C, C], BF, kind="ExternalInput").ap()
    w0b_d = nc.dram_tensor("w0b", [1, C], BF, kind="ExternalInput").ap()
    msk_d = nc.dram_tensor("msk", [64, 128], F32, kind="ExternalInput").ap()
    y_d = nc.dram_tensor("y", [C, NQ], F32, kind="ExternalOutput").ap()

    with tile.TileContext(nc) as tc:
        _body(tc, x_d, wq_d, bq_d, wk_d, bk_d, wv_d, w0_d, w0b_d, msk_d, y_d)
    nc.compile()
    return nc


_CACHE = {}


def _get_program():
    if "nc" not in _CACHE:
        _CACHE["nc"] = build_program()
    return _CACHE["nc"]


def make_in_maps(x, Wqkv, bqkv, W0, b0):
    f = np.float32
    x = np.asarray(x, f)
    Wqkv = np.asarray(Wqkv, f)
    bqkv = np.asarray(bqkv, f)
    W0 = np.asarray(W0, f)
    b0 = np.asarray(b0, f)

    scale = f(D) ** f(-0.5)
    # channel o = d*24 + k*8 + m ; column layout is head-major (m, d) -> m*32+d
    md = (np.arange(HEADS)[:, None] + 24 * np.arange(D)[None, :]).reshape(-1)
    q_rows, k_rows, v_rows = md + 0, md + 8, md + 16

    wq = np.ascontiguousarray((Wqkv[q_rows, :] * scale).T).astype(NPBF)
    bq = np.ascontiguousarray((bqkv[q_rows] * scale).reshape(-1, 1), dtype=f)
    wk = np.ascontiguousarray(Wqkv[k_rows, :].T).astype(NPBF)
    bk = np.ascontiguousarray(bqkv[k_rows].reshape(-1, 1), dtype=f)

    wv = np.zeros((C + 1, NV), f)
    for m in range(HEADS):
        vr = v_rows[m * D:(m + 1) * D]
        wv[0:C, m * 33:m * 33 + 32] = Wqkv[vr, :].T
        wv[C, m * 33:m * 33 + 32] = bqkv[vr]
        wv[C, m * 33 + 32] = 1.0
    wv = wv.astype(NPBF)

    w0 = np.ascontiguousarray(W0.T).astype(NPBF)  # [c, o], c rows head-major
    w0b = np.ascontiguousarray(b0[None, :]).astype(NPBF)

    msk = np.zeros((64, 128), f)
    msk[0, 0:32] = 1.0
    msk[32, 64:96] = 1.0

    shared = {"wq": wq, "bq": bq, "wk": wk, "bk": bk, "wv": wv, "w0": w0,
              "w0b": w0b, "msk": msk}
    maps = []
    for b in range(B):
        xb = x[b].reshape(C, N)
        for half in range(2):
            if half == 0:
                xp = xb
            else:
                xp = np.concatenate([xb[:, NQ:], xb[:, :NQ]], axis=1)
            x_aug = np.concatenate([xp, np.ones((1, N), f)], axis=0)
            maps.append({"x": np.ascontiguousarray(x_aug).astype(NPBF), **shared})
    return maps


def assemble_output(ys):
    out = np.empty((B, C, N), np.float32)
    for b in range(B):
        out[b][:, 0:NQ] = ys[2 * b]
        out[b][:, NQ:] = ys[2 * b + 1]
    return out.reshape(B, C, HH, WW)


def run(inputs, trace=False):
    nc = _get_program()
    maps = make_in_maps(**inputs)
    res = bass_utils.run_bass_kernel_spmd(
        nc, maps, core_ids=list(range(NCORES)), trace=trace
    )
    ys = [res.results[c]["y"] for c in range(NCORES)]
    return assemble_output(ys), res.exec_time_ns


def kernel(**inputs):
    out, _ = run(inputs, trace=False)
    return out
